# revision 13
# baseline (speedup 1.0000x reference)
"""AttentiveConvLSTM on 8 TRN2 NeuronCores, fp8-accelerated gate/attention convs.

Problem: B=4, T=4, C=256, H=30, W=40, CA=256 (reference.py semantics).

Sharding: 8 cores = 4 batch samples x 2 output-channel halves.
core = 2*b + s. Within a pair (fixed sample b):
  - both cores hold the full input x[b] and full h (allgathered per step),
  - core s computes the s-th 128-channel half of every conv output,
  - the attention logit e is AllReduced within the pair,
  - c/h recurrence state for the core's 128 channels stays local; h is
    AllGathered within the pair after each step.

Precision plan (validated vs the fp32 reference on CPU):
  - Wa conv (attention h-side) and Wx conv (gates x~-side) run in fp8e4
    (e4m3, max 240) with MatmulPerfMode.DoubleRow: one matmul contracts
    both 128-channel k-tiles at 2 fp8/cell, roughly halving PE time.
  - Ua, Uh, Va convs stay fp16: the Uh.h path amplifies any injected
    quantization error ~2x per recurrence step (fp8 there gives 20%+
    final error), and Ua/Va sit right at the error budget.
  - Scales (all powers of 2, exact): fp8 weights are shipped as 16*W.
    Moving operands carry S_t = 16 (step 0, where h0=sum_t x_t is large)
    or 32 (steps 1+): x is shipped as S_t*x_t, h is stored as S_t*h, and
    x~ = a*(S_t*x). fp16 weights are shipped as 16*W too so both matmul
    flavors accumulate into one PSUM group at scale 16*S_t; epilogue
    activations descale with scale=1/(16*S_t).

Layout trick for DoubleRow: its moving AP must be [128, 2, N] (3D), but a
6-row conv window of a padded [32, 42] image is 4D. Instead each window is
taken as a FLAT contiguous 250-element slice (6 rows x 42 minus trailing
pad); 10 interleaved pad columns produce junk PSUM columns that epilogues
skip with a strided read. fp16 matmuls use the same flat layout (502
elements for 12-row chunks) so both write identical PSUM positions.
"""

import numpy as np
import ml_dtypes

import concourse.bass as bass
import concourse.tile as tile
from concourse import bacc, mybir
from concourse.tile import add_dep_helper
from concourse.bass_utils import run_bass_kernel_spmd

f32 = mybir.dt.float32
f16 = mybir.dt.float16
f8 = mybir.dt.float8e4
E4 = ml_dtypes.float8_e4m3
DR = mybir.MatmulPerfMode.DoubleRow

B, T, C, H, W = 4, 4, 256, 30, 40
P = 128
KT = C // P  # 2 k-tiles over input channels
Hp, Wp = H + 2, W + 2  # 32 x 42 padded spatial
HW = H * W  # 1200
NCH = 3  # epilogue chunks per conv output
CHUNKS = [(0, 12), (12, 12), (24, 6)]  # (row0, nrows)
NTAP = 9
GROUPS = [[0, 1], [2, 3], [4, 5], [6, 7]]
TANH = mybir.ActivationFunctionType.Tanh
SIG = mybir.ActivationFunctionType.Sigmoid
EXP = mybir.ActivationFunctionType.Exp
COPY = mybir.ActivationFunctionType.Copy

SW = 16.0  # weight scale (both fp8 and fp16 conv weights)
SM = [16.0, 32.0, 32.0, 32.0]  # per-step moving-operand scale (h0 is large)


def _flat_off(tap, r0):
    dy, dx = tap // 3, tap % 3
    return (dy + r0) * Wp + dx


def _flen(nr):
    # flat window length for nr output rows: nr*42 minus the 2 trailing pads
    return nr * Wp - 2


def _psview(ps, nr):
    """Strided [*, sub, row, col] view of a flat-window PSUM tile, junk cols
    skipped. ps is [P or 1, nsub*252]; returns [*, nsub, 6, 40]."""
    nsub = nr // 6
    v = ps.rearrange("p (s r w) -> p s r w", s=nsub, r=6, w=Wp)
    return v[:, :, :, 0:W]


def _cview(t, nr):
    """Matching [*, sub, row, col] view of a compact [*, nr*W] tile."""
    nsub = nr // 6
    return t.rearrange("p (s r w) -> p s r w", s=nsub, r=6, w=W)


def build_nc():
    nc = bacc.Bacc("TRN2", target_bir_lowering=False, debug=False, num_devices=8)

    x_d = nc.dram_tensor("x", [T, KT, P, Hp, Wp], f16, kind="ExternalInput").ap()
    h0_d = nc.dram_tensor("h0", [KT, P, Hp, Wp], f16, kind="ExternalInput").ap()
    h08_d = nc.dram_tensor("h08", [KT, P, Hp, Wp], f8, kind="ExternalInput").ap()
    wa_d = nc.dram_tensor("wa", [P, NTAP * KT * P], f8, kind="ExternalInput").ap()
    ua_d = nc.dram_tensor("ua", [P, NTAP * KT * P], f16, kind="ExternalInput").ap()
    va_d = nc.dram_tensor("va", [P, NTAP], f16, kind="ExternalInput").ap()
    wx_d = nc.dram_tensor("wx", [P, NTAP * KT * 512], f8, kind="ExternalInput").ap()
    uh_d = nc.dram_tensor("uh", [P, NTAP * KT * 512], f16, kind="ExternalInput").ap()
    ab_d = nc.dram_tensor("ab", [P, 1], f32, kind="ExternalInput").ap()
    gb_d = nc.dram_tensor("gb", [P, 4], f32, kind="ExternalInput").ap()
    csel_d = nc.dram_tensor("csel", [KT], f32, kind="ExternalInput").ap()
    hout_d = nc.dram_tensor("hout", [P, HW], f32, kind="ExternalOutput").ap()

    # warm-up buffers: absorb first-collective latency during startup
    warm_in = nc.dram_tensor("warm_in", [1, 16], f32).ap()
    warm_out = nc.dram_tensor("warm_out", [1, 16], f32).ap()

    # internal DRAM bounce buffers for the per-step collectives
    e_part, e_full, h_half, h_full = [], [], [], []
    for t in range(T):
        e_part.append(nc.dram_tensor(f"e_part{t}", [1, HW], f32).ap())
        e_full.append(nc.dram_tensor(f"e_full{t}", [1, HW], f32).ap())
    # two gathers per step: chunk 0 (rows 0-11, launches early) and
    # chunks 1+2 merged (rows 12-29) — fewer serialized CC turnarounds
    GCH = [(0, 12), (12, 18)]
    for t in range(T - 1):
        h_half.append(
            [nc.dram_tensor(f"h_half{t}_{j}", [P, GCH[j][1] * W], f16).ap() for j in range(2)]
        )
        h_full.append(
            [
                nc.dram_tensor(f"h_full{t}_{j}", [KT, P, GCH[j][1] * W], f16).ap()
                for j in range(2)
            ]
        )

    with tile.TileContext(nc) as tc:
        import contextlib

        with contextlib.ExitStack() as ctx:
            wpool = ctx.enter_context(tc.tile_pool(name="wpool", bufs=1))
            state = ctx.enter_context(tc.tile_pool(name="state", bufs=1))
            xpool = ctx.enter_context(tc.tile_pool(name="xpool", bufs=3))
            psA = ctx.enter_context(tc.tile_pool(name="psA", bufs=4, space="PSUM"))
            psG = ctx.enter_context(tc.tile_pool(name="psG", bufs=4, space="PSUM"))
            ep = ctx.enter_context(tc.tile_pool(name="ep", bufs=8))
            erow = ctx.enter_context(tc.tile_pool(name="erow", bufs=2))

            # ---- load weights ----
            wa_sb = wpool.tile([P, NTAP * KT, P], f8)
            ua_sb = wpool.tile([P, NTAP * KT, P], f16)
            va_sb = wpool.tile([P, NTAP], f16)
            wx_sb = wpool.tile([P, NTAP * KT, 512], f8)
            uh_sb = wpool.tile([P, NTAP * KT, 512], f16)
            ab_sb = wpool.tile([P, 1], f32)
            gb_sb = wpool.tile([P, 4], f32)
            csel_sb = wpool.tile([P, KT], f32)
            ones_sb = wpool.tile([1, P], f16)
            # ua first on sync: it gates the very first matmuls; split in two
            # so the first taps' matmuls can start before the full load lands
            ua_r = ua_d.rearrange("p (i m) -> p i m", m=P)
            nc.sync.dma_start(out=ua_sb[:, 0:9, :], in_=ua_r[:, 0:9, :])
            nc.sync.dma_start(out=ua_sb[:, 9:18, :], in_=ua_r[:, 9:18, :])
            wz = wpool.tile([1, 16], f32)
            nc.vector.memset(wz, 0.0)
            nc.sync.dma_start(out=warm_in, in_=wz)
            nc.gpsimd.collective_compute(
                "AllReduce", mybir.AluOpType.add, replica_groups=GROUPS,
                ins=[warm_in], outs=[warm_out],
            )

            nc.sync.dma_start(out=ab_sb, in_=ab_d)
            nc.sync.dma_start(out=gb_sb, in_=gb_d)
            nc.sync.dma_start(out=va_sb, in_=va_d)
            nc.scalar.dma_start(out=wa_sb, in_=wa_d.rearrange("p (i m) -> p i m", m=P))
            # h0/h08 before the big uh/wx loads: step 0's Wa needs h08 ~12us in
            nc.sync.dma_start(out=h8_pad, in_=h08_d.rearrange("kt p h w -> p kt h w"))
            nc.scalar.dma_start(out=h_pad, in_=h0_d.rearrange("kt p h w -> p kt h w"))
            nc.sync.dma_start(out=uh_sb, in_=uh_d.rearrange("p (i m) -> p i m", m=512))
            nc.scalar.dma_start(out=wx_sb, in_=wx_d.rearrange("p (i m) -> p i m", m=512))
            nc.sync.dma_start(
                out=csel_sb,
                in_=bass.AP(
                    tensor=csel_d.tensor,
                    offset=csel_d.offset,
                    ap=[[0, P], [1, KT]],
                ),
            )
            nc.vector.memset(ones_sb, 1.0)

            # ---- persistent state tiles ----
            h_pad = state.tile([P, KT, Hp, Wp], f16)   # holds S_t * h
            h8_pad = state.tile([P, KT, Hp, Wp], f8)   # fp8 copy for Wa conv
            xtld8 = state.tile([P, KT, Hp, Wp], f8)    # x~ = a * (S_t*x), fp8
            t_pad = state.tile([P, Hp, Wp], f16)
            c_a = state.tile([P, HW], f32)
            c_b = state.tile([P, HW], f32)
            nc.vector.memset(xtld8, 0.0)
            nc.vector.memset(t_pad, 0.0)

            h_padf = h_pad.rearrange("p kt h w -> p kt (h w)")
            h8_padf = h8_pad.rearrange("p kt h w -> p kt (h w)")
            xtld8f = xtld8.rearrange("p kt h w -> p kt (h w)")
            t_padf = t_pad.rearrange("p h w -> p (h w)")

            # ---- stream x_t in; h0 = sum_t x_t (x arrives as S_t * x_t) ----
            def load_xt(t, pfx="s"):
                xt = xpool.tile([P, KT, Hp, Wp], f16, tag="xt", name=f"xt_{pfx}{t}")
                nc.gpsimd.dma_start(
                    out=xt, in_=x_d[t].rearrange("kt p h w -> p kt h w")
                )
                return xt

            # h0 (= 16*sum_t x_t, fp16+fp8) is precomputed on the host: its
            # DMAs run in parallel with the x/weight loads, so step 0's Wa
            # matmuls are not serialized behind an on-chip reduction
            xt_s0 = load_xt(0)

            # c0 = own kt-half of h0 (unscaled): csel carries the 1/16
            c_hw = c_a.rearrange("p (h w) -> p h w", h=H)
            c0t = ep.tile([P, H, W], f32, tag="c0t", bufs=1, name="c0t")
            nc.vector.tensor_scalar_mul(c0t, h_pad[:, 0, 1 : 1 + H, 1 : 1 + W],
                                        csel_sb[:, 0:1])
            nc.vector.tensor_scalar_mul(c_hw, h_pad[:, 1, 1 : 1 + H, 1 : 1 + W],
                                        csel_sb[:, 1:2])
            nc.vector.tensor_add(c_hw, c_hw, c0t)

            # Ua-side attention accumulation (fp16 flat windows): depends only
            # on x, so step t+1's Ua matmuls are emitted during step t's
            # e-roundtrip to keep the PE fed while the AllReduce is in flight
            def emit_ua(t, xt):
                xtf = xt.rearrange("p kt h w -> p kt (h w)")
                aps = []
                for j in range(NCH):
                    r0, nr = CHUNKS[j]
                    nsub = nr // 6
                    ps = psA.tile([P, nsub * 252], f32, tag="psA",
                                  name=f"aps{t}_{j}")
                    aps.append(ps)
                    fl = _flen(nr)
                    first = True
                    for tap in range(NTAP):
                        off = _flat_off(tap, r0)
                        for kt in range(KT):
                            i = tap * KT + kt
                            nc.tensor.matmul(
                                ps[:, 0:fl],
                                ua_sb[:, i, :],
                                xtf[:, kt, off : off + fl],
                                start=first,
                                stop=False,
                            )
                            first = False
                return aps

            xt_cur = xt_s0
            apsum_cur = emit_ua(0, xt_s0)

            # ---- time steps ----
            for t in range(T):
                xt = xt_cur
                xtf = xt.rearrange("p kt h w -> p kt (h w)")
                apsum = apsum_cur
                cin = c_a if t % 2 == 0 else c_b
                cout = c_b if t % 2 == 0 else c_a
                esc = 1.0 / (SW * SM[t])  # epilogue descale

                # 1) Wa conv on h8 (fp8 DoubleRow, 6-row subchunks)
                for j in range(NCH):
                    r0, nr = CHUNKS[j]
                    nsub = nr // 6
                    for s in range(nsub):
                        for tap in range(NTAP):
                            off = _flat_off(tap, r0 + 6 * s)
                            nc.tensor.matmul(
                                apsum[j][:, s * 252 : s * 252 + 250],
                                wa_sb[:, 2 * tap : 2 * tap + 2, :],
                                h8_padf[:, :, off : off + 250],
                                start=False,
                                stop=(s == nsub - 1 and tap == NTAP - 1),
                                perf_mode=DR,
                            )

                # 2) tanh into t_pad interior
                for j in range(NCH):
                    r0, nr = CHUNKS[j]
                    tview = t_pad[:, 1 + r0 : 1 + r0 + nr, 1 : 1 + W].rearrange(
                        "p (s x) w -> p s x w", x=6
                    )
                    nc.scalar.activation(
                        out=tview,
                        in_=_psview(apsum[j], nr),
                        func=TANH,
                        bias=ab_sb,
                        scale=esc,
                    )

                # 3) Va conv (fp16 flat windows) -> partial e
                e_sb = erow.tile([1, HW], f32, tag="erow", name=f"e_sb{t}")
                last_va = None
                for j in range(NCH):
                    r0, nr = CHUNKS[j]
                    nsub = nr // 6
                    fl = _flen(nr)
                    eps = psA.tile([1, nsub * 252], f32, tag="psA", name=f"eps{t}_{j}")
                    for tap in range(NTAP):
                        off = _flat_off(tap, r0)
                        last_va = nc.tensor.matmul(
                            eps[:, 0:fl],
                            va_sb[:, tap : tap + 1],
                            t_padf[:, off : off + fl],
                            start=(tap == 0),
                            stop=(tap == NTAP - 1),
                        )
                    nc.scalar.copy(
                        out=_cview(e_sb[:, r0 * W : (r0 + nr) * W], nr),
                        in_=_psview(eps, nr),
                    )

                # preload the EXP table while the AllReduce is in flight (the
                # attention tanhs above evicted it)
                dummy = ep.tile([1, 1], f32, tag="sc", name=f"dummy{t}")
                nc.scalar.activation(out=dummy, in_=ab_sb[0:1, 0:1], func=EXP)

                # 4) AllReduce e within the pair
                nc.sync.dma_start(out=e_part[t], in_=e_sb)
                nc.gpsimd.collective_compute(
                    "AllReduce",
                    mybir.AluOpType.add,
                    replica_groups=GROUPS,
                    ins=[e_part[t]],
                    outs=[e_full[t]],
                )
                ef_sb = erow.tile([1, HW], f32, tag="erow", name=f"ef_sb{t}")
                nc.sync.dma_start(out=ef_sb, in_=e_full[t])

                # prefetch next step's x and pre-run its Ua matmuls while the
                # e AllReduce is in flight
                if t < T - 1:
                    xt_cur = load_xt(t + 1)
                    apsum_cur = emit_ua(t + 1, xt_cur)

                # 5) pre-start Uh gate matmuls (fp16 flat windows; fills the PE
                # during the e roundtrip). Pinned after the last Va matmul so
                # the scheduler cannot hoist them ahead of the e chain.
                gpsums = [[None] * 4 for _ in range(NCH)]
                for j in range(NCH):
                    r0, nr = CHUNKS[j]
                    nsub = nr // 6
                    fl = _flen(nr)
                    for g in range(4):
                        ps = psG.tile([P, nsub * 252], f32, tag="psG",
                                      name=f"gps{t}_{j}_{g}")
                        gpsums[j][g] = ps
                        first = True
                        for tap in range(NTAP):
                            off = _flat_off(tap, r0)
                            for kt in range(KT):
                                i = tap * KT + kt
                                mm = nc.tensor.matmul(
                                    ps[:, 0:fl],
                                    uh_sb[:, i, g * P : (g + 1) * P],
                                    h_padf[:, kt, off : off + fl],
                                    start=first,
                                    stop=False,
                                )
                                if first and last_va is not None:
                                    add_dep_helper(mm.ins, last_va.ins, sync=False,
                                                   reason="uh after e-chain")
                                first = False

                # 6) softmax over the full spatial map (e is bounded by ~8,
                # so no max-subtraction is needed)
                pexp = erow.tile([1, HW], f32, tag="erow", name=f"pexp{t}")
                ssum = ep.tile([1, 1], f32, tag="sc", name=f"ssum{t}")
                nc.scalar.activation(
                    out=pexp, in_=ef_sb, func=EXP, accum_out=ssum
                )
                rinv = ep.tile([1, 1], f32, tag="sc", name=f"rinv{t}")
                nc.vector.reciprocal(out=rinv, in_=ssum)
                a_row = erow.tile([1, HW], f16, tag="erow", name=f"a_row{t}")
                nc.vector.tensor_scalar_mul(a_row, pexp, rinv)

                # 7) broadcast a over partitions; x~ = (S_t x) * a -> fp8
                for j in range(NCH):
                    r0, nr = CHUNKS[j]
                    bps = psA.tile([P, nr * W], f32, tag="psA", name=f"bps{t}_{j}")
                    nc.tensor.matmul(
                        bps,
                        ones_sb,
                        a_row[:, r0 * W : (r0 + nr) * W],
                        start=True,
                        stop=True,
                    )
                    for kt in range(KT):
                        nc.vector.tensor_mul(
                            xtld8[:, kt, 1 + r0 : 1 + r0 + nr, 1 : 1 + W],
                            xt[:, kt, 1 + r0 : 1 + r0 + nr, 1 : 1 + W],
                            bps.rearrange("p (h w) -> p h w", h=nr),
                        )

                # 8) finish gates with Wx on x~ (fp8 DoubleRow); per-chunk
                # epilogue
                hh12 = (
                    ep.tile([P, 18 * W], f16, tag="hh", bufs=2, name=f"hh12_{t}")
                    if t < T - 1 else None
                )
                for j in range(NCH):
                    r0, nr = CHUNKS[j]
                    nsub = nr // 6
                    jsl = slice(r0 * W, (r0 + nr) * W)
                    for g in range(4):
                        for s in range(nsub):
                            for tap in range(NTAP):
                                off = _flat_off(tap, r0 + 6 * s)
                                nc.tensor.matmul(
                                    gpsums[j][g][:, s * 252 : s * 252 + 250],
                                    wx_sb[:, 2 * tap : 2 * tap + 2,
                                          g * P : (g + 1) * P],
                                    xtld8f[:, :, off : off + 250],
                                    start=False,
                                    stop=(s == nsub - 1 and tap == NTAP - 1),
                                    perf_mode=DR,
                                )
                    i_c = ep.tile([P, nr * W], f32, tag="ep", name=f"i{t}_{j}")
                    f_c = ep.tile([P, nr * W], f32, tag="ep", name=f"f{t}_{j}")
                    tgc = ep.tile([P, nr * W], f32, tag="ep", name=f"tgc{t}_{j}")
                    o_c = ep.tile([P, nr * W], f32, tag="ep", name=f"o{t}_{j}")
                    for gi_, dst, fn in ((0, i_c, SIG), (1, f_c, SIG),
                                         (2, tgc, TANH), (3, o_c, SIG)):
                        nc.scalar.activation(
                            out=_cview(dst, nr),
                            in_=_psview(gpsums[j][gi_], nr),
                            func=fn,
                            bias=gb_sb[:, gi_ : gi_ + 1],
                            scale=esc,
                        )
                    t1 = ep.tile([P, nr * W], f32, tag="ep", name=f"t1_{t}_{j}")
                    t2 = ep.tile([P, nr * W], f32, tag="ep", name=f"t2_{t}_{j}")
                    nc.vector.tensor_mul(t1, f_c, cin[:, jsl])
                    nc.vector.tensor_mul(t2, i_c, tgc)
                    nc.vector.tensor_add(cout[:, jsl], t1, t2)
                    th = ep.tile([P, nr * W], f32, tag="ep", name=f"th{t}_{j}")
                    nc.scalar.activation(out=th, in_=cout[:, jsl], func=TANH)
                    if t < T - 1:
                        # h is stored scaled: h_half = (32*o)*th
                        o_s = ep.tile([P, nr * W], f32, tag="ep", name=f"os{t}_{j}")
                        nc.vector.tensor_scalar_mul(o_s, o_c, 32.0)
                        if j == 0:
                            h_new = ep.tile([P, nr * W], f16, tag="ep",
                                            name=f"hn{t}_{j}")
                            nc.vector.tensor_mul(h_new, o_s, th)
                            gat = (0, 0, 12, h_new)
                        else:
                            # chunks 1+2 share one gather payload
                            nc.vector.tensor_mul(
                                hh12[:, (r0 - 12) * W : (r0 - 12 + nr) * W],
                                o_s, th,
                            )
                            gat = (1, 12, 18, hh12) if j == 2 else None
                        if gat is not None:
                            gi2, gr0, gnr, gsrc = gat
                            nc.gpsimd.dma_start(out=h_half[t][gi2], in_=gsrc)
                            nc.gpsimd.collective_compute(
                                "AllGather",
                                mybir.AluOpType.bypass,
                                replica_groups=GROUPS,
                                ins=[h_half[t][gi2]],
                                outs=[h_full[t][gi2]],
                            )
                            # DMA-backs on sync so the gpsimd collective queue
                            # stays free; one fp8 shadow copy per gather
                            for kt in range(KT):
                                nc.sync.dma_start(
                                    out=h_pad[:, kt, 1 + gr0 : 1 + gr0 + gnr,
                                              1 : 1 + W],
                                    in_=h_full[t][gi2][kt].rearrange(
                                        "p (h w) -> p h w", h=gnr
                                    ),
                                )
                            nc.scalar.copy(
                                out=h8_pad[:, :, 1 + gr0 : 1 + gr0 + gnr, 1 : 1 + W],
                                in_=h_pad[:, :, 1 + gr0 : 1 + gr0 + gnr, 1 : 1 + W],
                            )
                    else:
                        ho = ep.tile([P, nr * W], f32, tag="ep", name=f"ho{t}_{j}")
                        nc.vector.tensor_mul(ho, o_c, th)
                        nc.sync.dma_start(out=hout_d[:, jsl], in_=ho)

    nc.compile()
    return nc


def prepare_in_maps(x, Wa, ba, Ua, bua, Va, Wx, bx, Uh, bh):
    """Shard + pre-transform the full inputs into per-core in_maps."""
    x = np.asarray(x, dtype=np.float32)
    xr = np.zeros((B, T, KT, P, Hp, Wp), dtype=np.float16)
    for t in range(T):
        xr[:, t, ..., 1 : 1 + H, 1 : 1 + W] = (
            (x[:, t] * SM[t]).astype(np.float16).reshape(B, KT, P, H, W)
        )
    # h0 = SM[0] * sum_t x_t, shipped padded in fp16 and fp8
    h0p = np.zeros((B, KT, P, Hp, Wp), dtype=np.float16)
    h0p[..., 1 : 1 + H, 1 : 1 + W] = (
        (x.sum(axis=1) * SM[0]).astype(np.float16).reshape(B, KT, P, H, W)
    )
    h08p = h0p.astype(E4)
    assert np.isfinite(h08p.astype(np.float32)).all()

    def conv_lhsT(Wf, s, m):
        # Wf: [3,3,C,COUT] HWIO; take columns for half s -> [P, NTAP*KT*m]
        sl = Wf.reshape(9, KT, P, Wf.shape[-1])[:, :, :, s * m : (s + 1) * m]
        return np.ascontiguousarray(
            sl.transpose(2, 0, 1, 3).reshape(P, NTAP * KT * m)
        )

    maps = []
    for b in range(B):
        for s in range(2):
            wa = conv_lhsT(np.asarray(Wa, np.float32) * SW, s, P)
            ua = conv_lhsT(np.asarray(Ua, np.float32) * SW, s, P)
            va = (
                np.asarray(Va, np.float32)[:, :, s * P : (s + 1) * P, 0]
                .reshape(9, P)
                .T.copy()
            )
            cols = np.concatenate(
                [np.arange(g * C + s * P, g * C + (s + 1) * P) for g in range(4)]
            )
            wx = (
                (np.asarray(Wx, np.float32) * SW)[..., cols]
                .reshape(9, KT, P, 512)
                .transpose(2, 0, 1, 3)
                .reshape(P, NTAP * KT * 512)
            )
            uh = (
                (np.asarray(Uh, np.float32) * SW)[..., cols]
                .reshape(9, KT, P, 512)
                .transpose(2, 0, 1, 3)
                .reshape(P, NTAP * KT * 512)
            )
            ab = (np.asarray(ba, np.float32) + np.asarray(bua, np.float32))[
                s * P : (s + 1) * P
            ].reshape(P, 1)
            gb = (np.asarray(bx, np.float32) + np.asarray(bh, np.float32))[cols]
            gb = gb.reshape(4, P).T.copy()
            csel = np.zeros(KT, np.float32)
            csel[s] = 1.0 / SM[0]  # c0 = h_pad/16 for the owned half
            wa8 = wa.astype(E4)
            wx8 = wx.astype(E4)
            assert np.isfinite(wa8.astype(np.float32)).all()
            assert np.isfinite(wx8.astype(np.float32)).all()
            maps.append(
                {
                    "x": np.ascontiguousarray(xr[b]),
                    "h0": np.ascontiguousarray(h0p[b]),
                    "h08": np.ascontiguousarray(h08p[b]),
                    "wa": wa8,
                    "ua": ua.astype(np.float16),
                    "va": np.ascontiguousarray(va).astype(np.float16),
                    "wx": wx8,
                    "uh": np.ascontiguousarray(uh).astype(np.float16),
                    "ab": ab,
                    "gb": np.ascontiguousarray(gb),
                    "csel": csel,
                }
            )
    return maps


_NC_CACHE = []


def get_nc():
    if not _NC_CACHE:
        _NC_CACHE.append(build_nc())
    return _NC_CACHE[0]


def assemble_output(results):
    out = np.empty((B, C, H, W), dtype=np.float32)
    for b in range(B):
        for s in range(2):
            out[b, s * P : (s + 1) * P] = results[2 * b + s]["hout"].reshape(P, H, W)
    return out


def kernel(x, Wa, ba, Ua, bua, Va, Wx, bx, Uh, bh):
    in_maps = prepare_in_maps(x, Wa, ba, Ua, bua, Va, Wx, bx, Uh, bh)
    nc = get_nc()
    res = run_bass_kernel_spmd(nc, in_maps, core_ids=list(range(8)))
    return assemble_output(res.results)


# revision 18
# speedup vs baseline: 1.1973x; 1.1973x over previous
"""AttentiveConvLSTM on 8 TRN2 NeuronCores, fp8-accelerated gate/attention convs.

Problem: B=4, T=4, C=256, H=30, W=40, CA=256 (reference.py semantics).

Sharding: 8 cores = 4 batch samples x 2 output-channel halves.
core = 2*b + s. Within a pair (fixed sample b):
  - both cores hold the full input x[b] and full h (allgathered per step),
  - core s computes the s-th 128-channel half of every conv output,
  - the attention logit e is AllReduced within the pair,
  - c/h recurrence state for the core's 128 channels stays local; h is
    AllGathered within the pair after each step.

Precision plan (validated vs the fp32 reference on CPU):
  - Wa conv (attention h-side) and Wx conv (gates x~-side) run in fp8e4
    (e4m3, max 240) with MatmulPerfMode.DoubleRow: one matmul contracts
    both 128-channel k-tiles at 2 fp8/cell, roughly halving PE time.
  - Ua, Uh, Va convs stay fp16: the Uh.h path amplifies any injected
    quantization error ~2x per recurrence step (fp8 there gives 20%+
    final error), and Ua/Va sit right at the error budget.
  - Scales (all powers of 2, exact): fp8 weights are shipped as 16*W.
    Moving operands carry S_t = 16 (step 0, where h0=sum_t x_t is large)
    or 32 (steps 1+): x is shipped as S_t*x_t, h is stored as S_t*h, and
    x~ = a*(S_t*x). fp16 weights are shipped as 16*W too so both matmul
    flavors accumulate into one PSUM group at scale 16*S_t; epilogue
    activations descale with scale=1/(16*S_t).

Layout trick for DoubleRow: its moving AP must be [128, 2, N] (3D), but a
6-row conv window of a padded [32, 42] image is 4D. Instead each window is
taken as a FLAT contiguous 250-element slice (6 rows x 42 minus trailing
pad); 10 interleaved pad columns produce junk PSUM columns that epilogues
skip with a strided read. fp16 matmuls use the same flat layout (502
elements for 12-row chunks) so both write identical PSUM positions.
"""

import numpy as np
import ml_dtypes

import concourse.bass as bass
import concourse.tile as tile
from concourse import bacc, mybir
from concourse.tile import add_dep_helper
from concourse.bass_utils import run_bass_kernel_spmd

f32 = mybir.dt.float32
f16 = mybir.dt.float16
f8 = mybir.dt.float8e4
E4 = ml_dtypes.float8_e4m3
DR = mybir.MatmulPerfMode.DoubleRow

B, T, C, H, W = 4, 4, 256, 30, 40
P = 128
KT = C // P  # 2 k-tiles over input channels
Hp, Wp = H + 2, W + 2  # 32 x 42 padded spatial
HW = H * W  # 1200
NCH = 3  # epilogue chunks per conv output
CHUNKS = [(0, 12), (12, 12), (24, 6)]  # (row0, nrows)
NTAP = 9
GROUPS = [[0, 1], [2, 3], [4, 5], [6, 7]]
TANH = mybir.ActivationFunctionType.Tanh
SIG = mybir.ActivationFunctionType.Sigmoid
EXP = mybir.ActivationFunctionType.Exp
COPY = mybir.ActivationFunctionType.Copy

SW = 16.0  # weight scale (both fp8 and fp16 conv weights)
SM = [16.0, 32.0, 32.0, 32.0]  # per-step moving-operand scale (h0 is large)


def _flat_off(tap, r0):
    dy, dx = tap // 3, tap % 3
    return (dy + r0) * Wp + dx


def _flen(nr):
    # flat window length for nr output rows: nr*42 minus the 2 trailing pads
    return nr * Wp - 2


def _psview(ps, nr):
    """Strided [*, sub, row, col] view of a flat-window PSUM tile, junk cols
    skipped. ps is [P or 1, nsub*252]; returns [*, nsub, 6, 40]."""
    nsub = nr // 6
    v = ps.rearrange("p (s r w) -> p s r w", s=nsub, r=6, w=Wp)
    return v[:, :, :, 0:W]


def _cview(t, nr):
    """Matching [*, sub, row, col] view of a compact [*, nr*W] tile."""
    nsub = nr // 6
    return t.rearrange("p (s r w) -> p s r w", s=nsub, r=6, w=W)


def build_nc():
    nc = bacc.Bacc("TRN2", target_bir_lowering=False, debug=False, num_devices=8)

    x_d = nc.dram_tensor("x", [T, KT, P, Hp, Wp], f16, kind="ExternalInput").ap()
    h0_d = nc.dram_tensor("h0", [KT, P, Hp, Wp], f16, kind="ExternalInput").ap()
    h08_d = nc.dram_tensor("h08", [KT, P, Hp, Wp], f8, kind="ExternalInput").ap()
    wa_d = nc.dram_tensor("wa", [P, NTAP * KT * P], f8, kind="ExternalInput").ap()
    ua_d = nc.dram_tensor("ua", [P, NTAP * KT * P], f16, kind="ExternalInput").ap()
    va_d = nc.dram_tensor("va", [P, NTAP], f16, kind="ExternalInput").ap()
    wx_d = nc.dram_tensor("wx", [P, NTAP * KT * 512], f8, kind="ExternalInput").ap()
    uh_d = nc.dram_tensor("uh", [P, NTAP * KT * 512], f16, kind="ExternalInput").ap()
    ab_d = nc.dram_tensor("ab", [P, 1], f32, kind="ExternalInput").ap()
    gb_d = nc.dram_tensor("gb", [P, 4], f32, kind="ExternalInput").ap()
    csel_d = nc.dram_tensor("csel", [KT], f32, kind="ExternalInput").ap()
    hout_d = nc.dram_tensor("hout", [P, HW], f32, kind="ExternalOutput").ap()

    # warm-up buffers: absorb first-collective latency during startup
    warm_in = nc.dram_tensor("warm_in", [1, 16], f32).ap()
    warm_out = nc.dram_tensor("warm_out", [1, 16], f32).ap()

    # internal DRAM bounce buffers for the per-step collectives
    e_part, e_full, h_half, h_full = [], [], [], []
    for t in range(T):
        e_part.append(nc.dram_tensor(f"e_part{t}", [1, HW], f32).ap())
        e_full.append(nc.dram_tensor(f"e_full{t}", [1, HW], f32).ap())
    for t in range(T - 1):
        h_half.append(
            [nc.dram_tensor(f"h_half{t}_{j}", [P, CHUNKS[j][1] * W], f16).ap() for j in range(NCH)]
        )
        h_full.append(
            [
                nc.dram_tensor(f"h_full{t}_{j}", [KT, P, CHUNKS[j][1] * W], f16).ap()
                for j in range(NCH)
            ]
        )

    with tile.TileContext(nc) as tc:
        import contextlib

        with contextlib.ExitStack() as ctx:
            wpool = ctx.enter_context(tc.tile_pool(name="wpool", bufs=1))
            state = ctx.enter_context(tc.tile_pool(name="state", bufs=1))
            xpool = ctx.enter_context(tc.tile_pool(name="xpool", bufs=3))
            psA = ctx.enter_context(tc.tile_pool(name="psA", bufs=4, space="PSUM"))
            psG = ctx.enter_context(tc.tile_pool(name="psG", bufs=4, space="PSUM"))
            ep = ctx.enter_context(tc.tile_pool(name="ep", bufs=8))
            erow = ctx.enter_context(tc.tile_pool(name="erow", bufs=2))

            # ---- persistent state tiles (created first: the h0/h08 DMAs
            # below are interleaved with the weight loads) ----
            h_pad = state.tile([P, KT, Hp, Wp], f16)   # holds S_t * h
            h8_pad = state.tile([P, KT, Hp, Wp], f8)   # fp8 copy for Wa conv
            xtld8 = state.tile([P, KT, Hp, Wp], f8)    # x~ = a * (S_t*x), fp8
            t_pad = state.tile([P, Hp, Wp], f16)
            c_a = state.tile([P, HW], f32)
            c_b = state.tile([P, HW], f32)
            nc.vector.memset(xtld8, 0.0)
            nc.vector.memset(t_pad, 0.0)

            # ---- load weights ----
            wa_sb = wpool.tile([P, NTAP * KT, P], f8)
            ua_sb = wpool.tile([P, NTAP * KT, P], f16)
            va_sb = wpool.tile([P, NTAP], f16)
            wx_sb = wpool.tile([P, NTAP * KT, 512], f8)
            uh_sb = wpool.tile([P, NTAP * KT, 512], f16)
            ab_sb = wpool.tile([P, 1], f32)
            gb_sb = wpool.tile([P, 4], f32)
            csel_sb = wpool.tile([P, KT], f32)
            ones_sb = wpool.tile([1, P], f16)
            # ua first on sync: it gates the very first matmuls; split in two
            # so the first taps' matmuls can start before the full load lands
            ua_r = ua_d.rearrange("p (i m) -> p i m", m=P)
            nc.sync.dma_start(out=ua_sb[:, 0:9, :], in_=ua_r[:, 0:9, :])
            nc.sync.dma_start(out=ua_sb[:, 9:18, :], in_=ua_r[:, 9:18, :])
            wz = wpool.tile([1, 16], f32)
            nc.vector.memset(wz, 0.0)
            nc.sync.dma_start(out=warm_in, in_=wz)
            nc.gpsimd.collective_compute(
                "AllReduce", mybir.AluOpType.add, replica_groups=GROUPS,
                ins=[warm_in], outs=[warm_out],
            )

            nc.sync.dma_start(out=ab_sb, in_=ab_d)
            nc.sync.dma_start(out=gb_sb, in_=gb_d)
            nc.sync.dma_start(out=va_sb, in_=va_d)
            nc.scalar.dma_start(out=wa_sb, in_=wa_d.rearrange("p (i m) -> p i m", m=P))
            # h0/h08 before the big uh/wx loads: step 0's Wa needs h08 ~12us in
            nc.sync.dma_start(out=h8_pad, in_=h08_d.rearrange("kt p h w -> p kt h w"))
            nc.scalar.dma_start(out=h_pad, in_=h0_d.rearrange("kt p h w -> p kt h w"))
            nc.sync.dma_start(out=uh_sb, in_=uh_d.rearrange("p (i m) -> p i m", m=512))
            nc.scalar.dma_start(out=wx_sb, in_=wx_d.rearrange("p (i m) -> p i m", m=512))
            nc.sync.dma_start(
                out=csel_sb,
                in_=bass.AP(
                    tensor=csel_d.tensor,
                    offset=csel_d.offset,
                    ap=[[0, P], [1, KT]],
                ),
            )
            nc.vector.memset(ones_sb, 1.0)

            h_padf = h_pad.rearrange("p kt h w -> p kt (h w)")
            h8_padf = h8_pad.rearrange("p kt h w -> p kt (h w)")
            xtld8f = xtld8.rearrange("p kt h w -> p kt (h w)")
            t_padf = t_pad.rearrange("p h w -> p (h w)")

            # ---- stream x_t in; h0 = sum_t x_t (x arrives as S_t * x_t) ----
            def load_xt(t, pfx="s"):
                xt = xpool.tile([P, KT, Hp, Wp], f16, tag="xt", name=f"xt_{pfx}{t}")
                nc.gpsimd.dma_start(
                    out=xt, in_=x_d[t].rearrange("kt p h w -> p kt h w")
                )
                return xt

            # h0 (= 16*sum_t x_t, fp16+fp8) is precomputed on the host: its
            # DMAs run in parallel with the x/weight loads, so step 0's Wa
            # matmuls are not serialized behind an on-chip reduction
            xt_s0 = load_xt(0)

            # c0 = own kt-half of h0 (unscaled): csel carries the 1/16
            c_hw = c_a.rearrange("p (h w) -> p h w", h=H)
            c0t = ep.tile([P, H, W], f32, tag="c0t", bufs=1, name="c0t")
            nc.vector.tensor_scalar_mul(c0t, h_pad[:, 0, 1 : 1 + H, 1 : 1 + W],
                                        csel_sb[:, 0:1])
            nc.vector.tensor_scalar_mul(c_hw, h_pad[:, 1, 1 : 1 + H, 1 : 1 + W],
                                        csel_sb[:, 1:2])
            nc.vector.tensor_add(c_hw, c_hw, c0t)

            # Ua-side attention accumulation (fp16 flat windows): depends only
            # on x, so step t+1's Ua matmuls are emitted during step t's
            # e-roundtrip to keep the PE fed while the AllReduce is in flight
            def emit_ua(t, xt):
                xtf = xt.rearrange("p kt h w -> p kt (h w)")
                aps = []
                for j in range(NCH):
                    r0, nr = CHUNKS[j]
                    nsub = nr // 6
                    ps = psA.tile([P, nsub * 252], f32, tag="psA",
                                  name=f"aps{t}_{j}")
                    aps.append(ps)
                    fl = _flen(nr)
                    first = True
                    for tap in range(NTAP):
                        off = _flat_off(tap, r0)
                        for kt in range(KT):
                            i = tap * KT + kt
                            nc.tensor.matmul(
                                ps[:, 0:fl],
                                ua_sb[:, i, :],
                                xtf[:, kt, off : off + fl],
                                start=first,
                                stop=False,
                            )
                            first = False
                return aps

            xt_cur = xt_s0
            apsum_cur = emit_ua(0, xt_s0)

            # ---- time steps ----
            for t in range(T):
                xt = xt_cur
                xtf = xt.rearrange("p kt h w -> p kt (h w)")
                apsum = apsum_cur
                cin = c_a if t % 2 == 0 else c_b
                cout = c_b if t % 2 == 0 else c_a
                esc = 1.0 / (SW * SM[t])  # epilogue descale

                # 1) Wa conv on h8 (fp8 DoubleRow, 6-row subchunks)
                for j in range(NCH):
                    r0, nr = CHUNKS[j]
                    nsub = nr // 6
                    for s in range(nsub):
                        for tap in range(NTAP):
                            off = _flat_off(tap, r0 + 6 * s)
                            nc.tensor.matmul(
                                apsum[j][:, s * 252 : s * 252 + 250],
                                wa_sb[:, 2 * tap : 2 * tap + 2, :],
                                h8_padf[:, :, off : off + 250],
                                start=False,
                                stop=(s == nsub - 1 and tap == NTAP - 1),
                                perf_mode=DR,
                            )

                # 2) tanh into t_pad interior
                for j in range(NCH):
                    r0, nr = CHUNKS[j]
                    tview = t_pad[:, 1 + r0 : 1 + r0 + nr, 1 : 1 + W].rearrange(
                        "p (s x) w -> p s x w", x=6
                    )
                    nc.scalar.activation(
                        out=tview,
                        in_=_psview(apsum[j], nr),
                        func=TANH,
                        bias=ab_sb,
                        scale=esc,
                    )

                # 3) Va conv (fp16 flat windows) -> partial e
                e_sb = erow.tile([1, HW], f32, tag="erow", name=f"e_sb{t}")
                last_va = None
                for j in range(NCH):
                    r0, nr = CHUNKS[j]
                    nsub = nr // 6
                    fl = _flen(nr)
                    eps = psA.tile([1, nsub * 252], f32, tag="psA", name=f"eps{t}_{j}")
                    for tap in range(NTAP):
                        off = _flat_off(tap, r0)
                        last_va = nc.tensor.matmul(
                            eps[:, 0:fl],
                            va_sb[:, tap : tap + 1],
                            t_padf[:, off : off + fl],
                            start=(tap == 0),
                            stop=(tap == NTAP - 1),
                        )
                    nc.scalar.copy(
                        out=_cview(e_sb[:, r0 * W : (r0 + nr) * W], nr),
                        in_=_psview(eps, nr),
                    )

                # preload the EXP table while the AllReduce is in flight (the
                # attention tanhs above evicted it)
                dummy = ep.tile([1, 1], f32, tag="sc", name=f"dummy{t}")
                nc.scalar.activation(out=dummy, in_=ab_sb[0:1, 0:1], func=EXP)

                # 4) AllReduce e within the pair
                nc.sync.dma_start(out=e_part[t], in_=e_sb)
                nc.gpsimd.collective_compute(
                    "AllReduce",
                    mybir.AluOpType.add,
                    replica_groups=GROUPS,
                    ins=[e_part[t]],
                    outs=[e_full[t]],
                )
                ef_sb = erow.tile([1, HW], f32, tag="erow", name=f"ef_sb{t}")
                nc.sync.dma_start(out=ef_sb, in_=e_full[t])

                # prefetch next step's x and pre-run its Ua matmuls while the
                # e AllReduce is in flight
                if t < T - 1:
                    xt_cur = load_xt(t + 1)
                    apsum_cur = emit_ua(t + 1, xt_cur)

                # 5) pre-start Uh gate matmuls (fp16 flat windows; fills the PE
                # during the e roundtrip). Pinned after the last Va matmul so
                # the scheduler cannot hoist them ahead of the e chain.
                gpsums = [[None] * 4 for _ in range(NCH)]
                for j in range(NCH):
                    r0, nr = CHUNKS[j]
                    nsub = nr // 6
                    fl = _flen(nr)
                    for g in range(4):
                        ps = psG.tile([P, nsub * 252], f32, tag="psG",
                                      name=f"gps{t}_{j}_{g}")
                        gpsums[j][g] = ps
                        first = True
                        for tap in range(NTAP):
                            off = _flat_off(tap, r0)
                            for kt in range(KT):
                                i = tap * KT + kt
                                mm = nc.tensor.matmul(
                                    ps[:, 0:fl],
                                    uh_sb[:, i, g * P : (g + 1) * P],
                                    h_padf[:, kt, off : off + fl],
                                    start=first,
                                    stop=False,
                                )
                                if first and last_va is not None:
                                    add_dep_helper(mm.ins, last_va.ins, sync=False,
                                                   reason="uh after e-chain")
                                first = False

                # 6) softmax over the full spatial map (e is bounded by ~8,
                # so no max-subtraction is needed)
                pexp = erow.tile([1, HW], f32, tag="erow", name=f"pexp{t}")
                ssum = ep.tile([1, 1], f32, tag="sc", name=f"ssum{t}")
                nc.scalar.activation(
                    out=pexp, in_=ef_sb, func=EXP, accum_out=ssum
                )
                rinv = ep.tile([1, 1], f32, tag="sc", name=f"rinv{t}")
                nc.vector.reciprocal(out=rinv, in_=ssum)
                a_row = erow.tile([1, HW], f16, tag="erow", name=f"a_row{t}")
                nc.vector.tensor_scalar_mul(a_row, pexp, rinv)

                # 7) broadcast a over partitions; x~ = (S_t x) * a -> fp8
                for j in range(NCH):
                    r0, nr = CHUNKS[j]
                    bps = psA.tile([P, nr * W], f32, tag="psA", name=f"bps{t}_{j}")
                    nc.tensor.matmul(
                        bps,
                        ones_sb,
                        a_row[:, r0 * W : (r0 + nr) * W],
                        start=True,
                        stop=True,
                    )
                    for kt in range(KT):
                        nc.vector.tensor_mul(
                            xtld8[:, kt, 1 + r0 : 1 + r0 + nr, 1 : 1 + W],
                            xt[:, kt, 1 + r0 : 1 + r0 + nr, 1 : 1 + W],
                            bps.rearrange("p (h w) -> p h w", h=nr),
                        )

                # 8) finish gates with Wx on x~ (fp8 DoubleRow); per-chunk
                # epilogue
                for j in range(NCH):
                    r0, nr = CHUNKS[j]
                    nsub = nr // 6
                    jsl = slice(r0 * W, (r0 + nr) * W)
                    for g in range(4):
                        for s in range(nsub):
                            for tap in range(NTAP):
                                off = _flat_off(tap, r0 + 6 * s)
                                nc.tensor.matmul(
                                    gpsums[j][g][:, s * 252 : s * 252 + 250],
                                    wx_sb[:, 2 * tap : 2 * tap + 2,
                                          g * P : (g + 1) * P],
                                    xtld8f[:, :, off : off + 250],
                                    start=False,
                                    stop=(s == nsub - 1 and tap == NTAP - 1),
                                    perf_mode=DR,
                                )
                    i_c = ep.tile([P, nr * W], f32, tag="ep", name=f"i{t}_{j}")
                    f_c = ep.tile([P, nr * W], f32, tag="ep", name=f"f{t}_{j}")
                    tgc = ep.tile([P, nr * W], f32, tag="ep", name=f"tgc{t}_{j}")
                    o_c = ep.tile([P, nr * W], f32, tag="ep", name=f"o{t}_{j}")
                    for gi_, dst, fn in ((0, i_c, SIG), (1, f_c, SIG),
                                         (2, tgc, TANH), (3, o_c, SIG)):
                        nc.scalar.activation(
                            out=_cview(dst, nr),
                            in_=_psview(gpsums[j][gi_], nr),
                            func=fn,
                            bias=gb_sb[:, gi_ : gi_ + 1],
                            scale=esc,
                        )
                    t1 = ep.tile([P, nr * W], f32, tag="ep", name=f"t1_{t}_{j}")
                    t2 = ep.tile([P, nr * W], f32, tag="ep", name=f"t2_{t}_{j}")
                    nc.vector.tensor_mul(t1, f_c, cin[:, jsl])
                    nc.vector.tensor_mul(t2, i_c, tgc)
                    nc.vector.tensor_add(cout[:, jsl], t1, t2)
                    th = ep.tile([P, nr * W], f32, tag="ep", name=f"th{t}_{j}")
                    nc.scalar.activation(out=th, in_=cout[:, jsl], func=TANH)
                    if t < T - 1:
                        # h is stored scaled: h_half = (32*o)*th
                        o_s = ep.tile([P, nr * W], f32, tag="ep", name=f"os{t}_{j}")
                        nc.vector.tensor_scalar_mul(o_s, o_c, 32.0)
                        h_new = ep.tile([P, nr * W], f16, tag="ep", name=f"hn{t}_{j}")
                        nc.vector.tensor_mul(h_new, o_s, th)
                        nc.gpsimd.dma_start(out=h_half[t][j], in_=h_new)
                        nc.gpsimd.collective_compute(
                            "AllGather",
                            mybir.AluOpType.bypass,
                            replica_groups=GROUPS,
                            ins=[h_half[t][j]],
                            outs=[h_full[t][j]],
                        )
                        # DMA-backs on sync so the gpsimd collective queue
                        # stays free; one merged fp8 shadow copy per chunk
                        for kt in range(KT):
                            nc.sync.dma_start(
                                out=h_pad[:, kt, 1 + r0 : 1 + r0 + nr, 1 : 1 + W],
                                in_=h_full[t][j][kt].rearrange(
                                    "p (h w) -> p h w", h=nr
                                ),
                            )
                        nc.scalar.copy(
                            out=h8_pad[:, :, 1 + r0 : 1 + r0 + nr, 1 : 1 + W],
                            in_=h_pad[:, :, 1 + r0 : 1 + r0 + nr, 1 : 1 + W],
                        )
                    else:
                        ho = ep.tile([P, nr * W], f32, tag="ep", name=f"ho{t}_{j}")
                        nc.vector.tensor_mul(ho, o_c, th)
                        nc.sync.dma_start(out=hout_d[:, jsl], in_=ho)

    nc.compile()
    return nc


def prepare_in_maps(x, Wa, ba, Ua, bua, Va, Wx, bx, Uh, bh):
    """Shard + pre-transform the full inputs into per-core in_maps."""
    x = np.asarray(x, dtype=np.float32)
    xr = np.zeros((B, T, KT, P, Hp, Wp), dtype=np.float16)
    for t in range(T):
        xr[:, t, ..., 1 : 1 + H, 1 : 1 + W] = (
            (x[:, t] * SM[t]).astype(np.float16).reshape(B, KT, P, H, W)
        )
    # h0 = SM[0] * sum_t x_t, shipped padded in fp16 and fp8
    h0p = np.zeros((B, KT, P, Hp, Wp), dtype=np.float16)
    h0p[..., 1 : 1 + H, 1 : 1 + W] = (
        (x.sum(axis=1) * SM[0]).astype(np.float16).reshape(B, KT, P, H, W)
    )
    h08p = h0p.astype(E4)
    assert np.isfinite(h08p.astype(np.float32)).all()

    def conv_lhsT(Wf, s, m):
        # Wf: [3,3,C,COUT] HWIO; take columns for half s -> [P, NTAP*KT*m]
        sl = Wf.reshape(9, KT, P, Wf.shape[-1])[:, :, :, s * m : (s + 1) * m]
        return np.ascontiguousarray(
            sl.transpose(2, 0, 1, 3).reshape(P, NTAP * KT * m)
        )

    maps = []
    for b in range(B):
        for s in range(2):
            wa = conv_lhsT(np.asarray(Wa, np.float32) * SW, s, P)
            ua = conv_lhsT(np.asarray(Ua, np.float32) * SW, s, P)
            va = (
                np.asarray(Va, np.float32)[:, :, s * P : (s + 1) * P, 0]
                .reshape(9, P)
                .T.copy()
            )
            cols = np.concatenate(
                [np.arange(g * C + s * P, g * C + (s + 1) * P) for g in range(4)]
            )
            wx = (
                (np.asarray(Wx, np.float32) * SW)[..., cols]
                .reshape(9, KT, P, 512)
                .transpose(2, 0, 1, 3)
                .reshape(P, NTAP * KT * 512)
            )
            uh = (
                (np.asarray(Uh, np.float32) * SW)[..., cols]
                .reshape(9, KT, P, 512)
                .transpose(2, 0, 1, 3)
                .reshape(P, NTAP * KT * 512)
            )
            ab = (np.asarray(ba, np.float32) + np.asarray(bua, np.float32))[
                s * P : (s + 1) * P
            ].reshape(P, 1)
            gb = (np.asarray(bx, np.float32) + np.asarray(bh, np.float32))[cols]
            gb = gb.reshape(4, P).T.copy()
            csel = np.zeros(KT, np.float32)
            csel[s] = 1.0 / SM[0]  # c0 = h_pad/16 for the owned half
            wa8 = wa.astype(E4)
            wx8 = wx.astype(E4)
            assert np.isfinite(wa8.astype(np.float32)).all()
            assert np.isfinite(wx8.astype(np.float32)).all()
            maps.append(
                {
                    "x": np.ascontiguousarray(xr[b]),
                    "h0": np.ascontiguousarray(h0p[b]),
                    "h08": np.ascontiguousarray(h08p[b]),
                    "wa": wa8,
                    "ua": ua.astype(np.float16),
                    "va": np.ascontiguousarray(va).astype(np.float16),
                    "wx": wx8,
                    "uh": np.ascontiguousarray(uh).astype(np.float16),
                    "ab": ab,
                    "gb": np.ascontiguousarray(gb),
                    "csel": csel,
                }
            )
    return maps


_NC_CACHE = []


def get_nc():
    if not _NC_CACHE:
        _NC_CACHE.append(build_nc())
    return _NC_CACHE[0]


def assemble_output(results):
    out = np.empty((B, C, H, W), dtype=np.float32)
    for b in range(B):
        for s in range(2):
            out[b, s * P : (s + 1) * P] = results[2 * b + s]["hout"].reshape(P, H, W)
    return out


def kernel(x, Wa, ba, Ua, bua, Va, Wx, bx, Uh, bh):
    in_maps = prepare_in_maps(x, Wa, ba, Ua, bua, Va, Wx, bx, Uh, bh)
    nc = get_nc()
    res = run_bass_kernel_spmd(nc, in_maps, core_ids=list(range(8)))
    return assemble_output(res.results)
